# revision 2
# baseline (speedup 1.0000x reference)
"""Trainium2 Bass kernel for the Capsule routing layer.

Math (see module docstring of the problem):
    R_nor = softmax(R[0], axis=0)                      # over N, per capsule c
    u[b,n,c,j] = sum_k W[0,n,c,j,k] * x[b,n,k]
    s[b,c,j]   = sum_n u[b,n,c,j] * R_nor[n,c]
    ss = sum_j s^2 + EPS ; out = sqrt(ss)/(1+ss) * s   # squash

Distribution: output-parallel over capsules C=32 across 8 cores (4 capsules
per core).  Each core performs the full contraction over (n,k) = 16384 for
its capsule slice, so there is no cross-core reduction (collectives on TRN2
have a ~20us latency floor which would dominate this ~20us kernel).

Per-core device algorithm:
    s[b,(c,j)] = sum_{(n,k)} x[b,n,k] * W[n,c,j,k] * exp(R[n,c]) / Z_c
    Z_c        = sum_n exp(R[n,c])
exp(R) is folded into W with one broadcast vector multiply per chunk, the
(n,k) contraction runs on the PE as 128 accumulating matmuls of
[p=128] x [m=32(b)] x [f=64(c,j)], and Z comes from the same exp(R) tile via
a free-dim reduce + ones-matmul partition reduce.  Softmax scale 1/Z and the
squash run on the tiny [32,64] result.

Host-side work is layout only (transpose/replicate of the raw inputs into
DMA-friendly shards); all arithmetic happens on device.
"""

import os

import numpy as np

P = 128                    # SBUF partitions
B, N, DIN, C, DOUT = 32, 2048, 8, 32, 16
NCORES = 8
CS = C // NCORES           # capsules per core (4)
NPB = P // DIN             # n's per 128-row block (16)
NBLK = (N * DIN) // P      # nk blocks of 128 (128)
BPC = 8                    # blocks per W chunk
CHUNKS = NBLK // BPC       # 16 chunks of [128, 512]
XT = 4                     # number of x tiles
FW = CS * DOUT             # free width per block (64)
EPS = 1e-7

LAST_EXEC_TIME_NS = None

_compiled = None


def _build(debug=False):
    from concourse import bacc, mybir, tile

    f32 = mybir.dt.float32
    Exp = mybir.ActivationFunctionType.Exp
    Sqrt = mybir.ActivationFunctionType.Sqrt
    AxX = mybir.AxisListType.X

    nc = bacc.Bacc(
        "TRN2", target_bir_lowering=False, debug=debug, num_devices=NCORES
    )
    wp = nc.dram_tensor("w_prep", [P, NBLK * FW], f32, kind="ExternalInput")
    xp = nc.dram_tensor("x_prep", [P, NBLK * B], f32, kind="ExternalInput")
    rp = nc.dram_tensor("r_rep", [P, NBLK * CS], f32, kind="ExternalInput")
    out = nc.dram_tensor("out", [B, FW], f32, kind="ExternalOutput")

    with tile.TileContext(nc) as tc:
        with (
            tc.tile_pool(name="wpool", bufs=3) as wpool,
            tc.tile_pool(name="misc", bufs=1) as misc,
            tc.tile_pool(name="ppool", bufs=1, space="PSUM") as ppool,
        ):
            # x and R loads go on the ACT HW-DGE ring so they don't queue
            # ahead of the W stream on the SP ring.
            xcols = NBLK * B // XT
            x_tiles = []
            for i in range(XT):
                xt_ = misc.tile([P, xcols], f32, tag=f"x{i}")
                nc.scalar.dma_start(out=xt_[:], in_=xp[:, i * xcols : (i + 1) * xcols])
                x_tiles.append(xt_)
            r_t = misc.tile([P, NBLK * CS], f32, tag="r")
            nc.scalar.dma_start(out=r_t[:], in_=rp[:])
            e_t = misc.tile([P, NBLK * CS], f32, tag="e")
            nc.scalar.activation(e_t[:], r_t[:], Exp)

            # Main contraction: for each chunk of 8 nk-blocks, stream W,
            # fold in exp(R[n,c]) (broadcast over j and the k-subrows), and
            # accumulate 8 matmuls into the [32, 64] PSUM tile.
            s_ps = ppool.tile([B, FW], f32, tag="s")
            for ch in range(CHUNKS):
                w_t = wpool.tile([P, BPC * FW], f32, tag="w")
                nc.sync.dma_start(
                    out=w_t[:], in_=wp[:, ch * BPC * FW : (ch + 1) * BPC * FW]
                )
                we_t = wpool.tile([P, BPC * FW], f32, tag="we")
                e_view = (
                    e_t[:, ch * BPC * CS : (ch + 1) * BPC * CS]
                    .rearrange("p (blk c) -> p blk c", blk=BPC, c=CS)
                    .unsqueeze(3)
                    .broadcast_to([P, BPC, CS, DOUT])
                )
                nc.vector.tensor_mul(
                    we_t[:].rearrange("p (blk c j) -> p blk c j", blk=BPC, c=CS, j=DOUT),
                    w_t[:].rearrange("p (blk c j) -> p blk c j", blk=BPC, c=CS, j=DOUT),
                    e_view,
                )
                for bi in range(BPC):
                    blk = ch * BPC + bi
                    xt_i, xcol = divmod(blk, NBLK // XT)
                    nc.tensor.matmul(
                        s_ps[:],
                        x_tiles[xt_i][:, xcol * B : (xcol + 1) * B],
                        we_t[:, bi * FW : (bi + 1) * FW],
                        start=(blk == 0),
                        stop=(blk == NBLK - 1),
                    )

            # Z_c = sum_n exp(R[n,c]).  Each n appears DIN times across the
            # partition rows, so reduce blocks on the free axis, then a
            # (1/DIN)-ones matmul reduces partitions.
            en_sum = misc.tile([P, CS], f32, tag="en_sum")
            nc.vector.reduce_sum(
                out=en_sum[:],
                in_=e_t[:].rearrange("p (blk c) -> p c blk", blk=NBLK, c=CS),
                axis=AxX,
            )
            inv_din = misc.tile([P, 1], f32, tag="inv_din")
            nc.vector.memset(inv_din[:], 1.0 / DIN)
            z_ps = ppool.tile([1, CS], f32, tag="z")
            nc.tensor.matmul(z_ps[:], inv_din[:], en_sum[:])
            invz = misc.tile([1, CS], f32, tag="invz")
            nc.vector.reciprocal(invz[:], z_ps[:])
            # Broadcast 1/Z to all 32 batch partitions with a rank-1 matmul.
            ones_b = misc.tile([1, B], f32, tag="ones_b")
            nc.vector.memset(ones_b[:], 1.0)
            bc_ps = ppool.tile([B, CS], f32, tag="bc")
            nc.tensor.matmul(bc_ps[:], ones_b[:], invz[:])
            bc_sb = misc.tile([B, CS], f32, tag="bc_sb")
            nc.scalar.copy(bc_sb[:], bc_ps[:])

            # s = s_unnorm / Z ; squash: out = sqrt(ss)/(1+ss) * s
            sn = misc.tile([B, FW], f32, tag="sn")
            nc.vector.tensor_mul(
                sn[:].rearrange("p (c j) -> p c j", c=CS, j=DOUT),
                s_ps[:].rearrange("p (c j) -> p c j", c=CS, j=DOUT),
                bc_sb[:].unsqueeze(2).broadcast_to([B, CS, DOUT]),
            )
            sq = misc.tile([B, FW], f32, tag="sq")
            nc.vector.tensor_mul(sq[:], sn[:], sn[:])
            ss = misc.tile([B, CS], f32, tag="ss")
            nc.vector.reduce_sum(
                out=ss[:],
                in_=sq[:].rearrange("p (c j) -> p c j", c=CS, j=DOUT),
                axis=AxX,
            )
            eps_t = misc.tile([B, 1], f32, tag="eps")
            nc.vector.memset(eps_t[:], EPS)
            sqrt_ss = misc.tile([B, CS], f32, tag="sqrt_ss")
            nc.scalar.activation(sqrt_ss[:], ss[:], Sqrt, bias=eps_t[:])
            den = misc.tile([B, CS], f32, tag="den")
            nc.vector.tensor_scalar_add(den[:], ss[:], 1.0 + EPS)
            rden = misc.tile([B, CS], f32, tag="rden")
            nc.vector.reciprocal(rden[:], den[:])
            scl = misc.tile([B, CS], f32, tag="scl")
            nc.vector.tensor_mul(scl[:], sqrt_ss[:], rden[:])
            o_t = misc.tile([B, FW], f32, tag="o")
            nc.vector.tensor_mul(
                o_t[:].rearrange("p (c j) -> p c j", c=CS, j=DOUT),
                sn[:].rearrange("p (c j) -> p c j", c=CS, j=DOUT),
                scl[:].unsqueeze(2).broadcast_to([B, CS, DOUT]),
            )
            nc.sync.dma_start(out=out[:], in_=o_t[:])

    nc.compile()
    return nc


def _prep(x, W, R):
    """Layout-only host prep: shard + transpose into DMA-friendly tiles.

    Row index everywhere: p = n_in_blk * DIN + k.
    """
    x = np.ascontiguousarray(x, dtype=np.float32)
    W = np.ascontiguousarray(W, dtype=np.float32)
    R = np.ascontiguousarray(R, dtype=np.float32)

    # x_prep[p, blk*B + b] = x[b, n(blk, p), k(p)]   (shared by all cores)
    x_prep = np.ascontiguousarray(
        x.reshape(B, NBLK, NPB, DIN).transpose(2, 3, 1, 0).reshape(P, NBLK * B)
    )

    w_maps, r_maps = [], []
    for i in range(NCORES):
        cs = slice(i * CS, (i + 1) * CS)
        # w_prep[p, blk*FW + c*DOUT + j] = W[0, n(blk,p), c, j, k(p)]
        Wc = W[0][:, cs]  # [N, CS, DOUT, DIN]
        w_maps.append(
            np.ascontiguousarray(
                Wc.reshape(NBLK, NPB, CS, DOUT, DIN)
                .transpose(1, 4, 0, 2, 3)
                .reshape(P, NBLK * FW)
            )
        )
        # r_rep[p, blk*CS + c] = R[0, n(blk,p), c]   (replicated over k)
        Rc = R[0][:, cs].reshape(NBLK, NPB, CS).transpose(1, 0, 2)
        r_maps.append(
            np.ascontiguousarray(
                np.broadcast_to(Rc[:, None], (NPB, DIN, NBLK, CS)).reshape(
                    P, NBLK * CS
                )
            )
        )
    return x_prep, w_maps, r_maps


def kernel(**inputs):
    global _compiled, LAST_EXEC_TIME_NS
    x, W, R = inputs["x"], inputs["W"], inputs["R"]
    if _compiled is None:
        _compiled = _build()
    nc = _compiled

    x_prep, w_maps, r_maps = _prep(np.asarray(x), np.asarray(W), np.asarray(R))
    in_maps = [
        {"w_prep": w_maps[i], "x_prep": x_prep, "r_rep": r_maps[i]}
        for i in range(NCORES)
    ]

    from concourse.bass_utils import run_bass_kernel_spmd

    trace = bool(os.environ.get("BASS_KERNEL_TRACE"))
    res = run_bass_kernel_spmd(nc, in_maps, list(range(NCORES)), trace=trace)
    LAST_EXEC_TIME_NS = res.exec_time_ns

    outs = [res.results[i]["out"].reshape(B, CS, DOUT) for i in range(NCORES)]
    return np.ascontiguousarray(np.concatenate(outs, axis=1))


# revision 3
# speedup vs baseline: 1.0784x; 1.0784x over previous
"""Trainium2 Bass kernel for the Capsule routing layer.

Math (see module docstring of the problem):
    R_nor = softmax(R[0], axis=0)                      # over N, per capsule c
    u[b,n,c,j] = sum_k W[0,n,c,j,k] * x[b,n,k]
    s[b,c,j]   = sum_n u[b,n,c,j] * R_nor[n,c]
    ss = sum_j s^2 + EPS ; out = sqrt(ss)/(1+ss) * s   # squash

Distribution: output-parallel over capsules C=32 across 8 cores (4 capsules
per core).  Each core performs the full contraction over (n,k) = 16384 for
its capsule slice, so there is no cross-core reduction (collectives on TRN2
have a ~20us latency floor which would dominate this ~20us kernel).

Per-core device algorithm:
    s[b,(c,j)] = sum_{(n,k)} x[b,n,k] * W[n,c,j,k] * exp(R[n,c]) / Z_c
    Z_c        = sum_n exp(R[n,c])
exp(R) is folded into W with one broadcast vector multiply per chunk, the
(n,k) contraction runs on the PE as 128 accumulating matmuls of
[p=128] x [m=32(b)] x [f=64(c,j)], and Z comes from the same exp(R) tile via
a free-dim reduce + ones-matmul partition reduce.  Softmax scale 1/Z and the
squash run on the tiny [32,64] result.

Host-side work is layout only (transpose/replicate of the raw inputs into
DMA-friendly shards); all arithmetic happens on device.
"""

import os

import numpy as np

P = 128                    # SBUF partitions
B, N, DIN, C, DOUT = 32, 2048, 8, 32, 16
NCORES = 8
CS = C // NCORES           # capsules per core (4)
NPB = P // DIN             # n's per 128-row block (16)
NBLK = (N * DIN) // P      # nk blocks of 128 (128)
BPC = 8                    # blocks per W chunk
CHUNKS = NBLK // BPC       # 16 chunks of [128, 512]
XT = 4                     # number of x tiles
FW = CS * DOUT             # free width per block (64)
EPS = 1e-7

LAST_EXEC_TIME_NS = None

_compiled = None


def _build(debug=False):
    from concourse import bacc, mybir, tile

    f32 = mybir.dt.float32
    Exp = mybir.ActivationFunctionType.Exp
    Sqrt = mybir.ActivationFunctionType.Sqrt
    AxX = mybir.AxisListType.X

    nc = bacc.Bacc(
        "TRN2", target_bir_lowering=False, debug=debug, num_devices=NCORES
    )
    wp = nc.dram_tensor("w_prep", [P, NBLK * FW], f32, kind="ExternalInput")
    xp = nc.dram_tensor("x_prep", [P, NBLK * B], f32, kind="ExternalInput")
    rp = nc.dram_tensor("r_rep", [P, NBLK * CS], f32, kind="ExternalInput")
    out = nc.dram_tensor("out", [B, FW], f32, kind="ExternalOutput")

    with tile.TileContext(nc) as tc:
        with (
            tc.tile_pool(name="wpool", bufs=3) as wpool,
            tc.tile_pool(name="misc", bufs=1) as misc,
            tc.tile_pool(name="ppool", bufs=1, space="PSUM") as ppool,
        ):
            # R + x loads go on the ACT HW-DGE ring so they don't queue
            # ahead of the W stream on the SP ring.  R goes absolutely first:
            # exp(R) gates every W-chunk multiply, and everything downstream.
            r_t = misc.tile([P, NBLK * CS], f32, tag="r")
            nc.scalar.dma_start(out=r_t[:], in_=rp[:])
            e_t = misc.tile([P, NBLK * CS], f32, tag="e")
            nc.scalar.activation(e_t[:], r_t[:], Exp)
            xcols = NBLK * B // XT
            x_tiles = []
            for i in range(XT):
                xt_ = misc.tile([P, xcols], f32, tag=f"x{i}")
                nc.scalar.dma_start(out=xt_[:], in_=xp[:, i * xcols : (i + 1) * xcols])
                x_tiles.append(xt_)

            # Main contraction: for each chunk of 8 nk-blocks, stream W,
            # fold in exp(R[n,c]) (broadcast over j and the k-subrows), and
            # accumulate 8 matmuls into the [32, 64] PSUM tile.
            s_ps = ppool.tile([B, FW], f32, tag="s")
            for ch in range(CHUNKS):
                w_t = wpool.tile([P, BPC * FW], f32, tag="w")
                nc.sync.dma_start(
                    out=w_t[:], in_=wp[:, ch * BPC * FW : (ch + 1) * BPC * FW]
                )
                we_t = wpool.tile([P, BPC * FW], f32, tag="we")
                e_view = (
                    e_t[:, ch * BPC * CS : (ch + 1) * BPC * CS]
                    .rearrange("p (blk c) -> p blk c", blk=BPC, c=CS)
                    .unsqueeze(3)
                    .broadcast_to([P, BPC, CS, DOUT])
                )
                nc.vector.tensor_mul(
                    we_t[:].rearrange("p (blk c j) -> p blk c j", blk=BPC, c=CS, j=DOUT),
                    w_t[:].rearrange("p (blk c j) -> p blk c j", blk=BPC, c=CS, j=DOUT),
                    e_view,
                )
                for bi in range(BPC):
                    blk = ch * BPC + bi
                    xt_i, xcol = divmod(blk, NBLK // XT)
                    nc.tensor.matmul(
                        s_ps[:],
                        x_tiles[xt_i][:, xcol * B : (xcol + 1) * B],
                        we_t[:, bi * FW : (bi + 1) * FW],
                        start=(blk == 0),
                        stop=(blk == NBLK - 1),
                    )

            # Z_c = sum_n exp(R[n,c]).  Each n appears DIN times across the
            # partition rows, so reduce blocks on the free axis, then a
            # (1/DIN)-ones matmul reduces partitions.
            en_sum = misc.tile([P, CS], f32, tag="en_sum")
            nc.vector.reduce_sum(
                out=en_sum[:],
                in_=e_t[:].rearrange("p (blk c) -> p c blk", blk=NBLK, c=CS),
                axis=AxX,
            )
            inv_din = misc.tile([P, 1], f32, tag="inv_din")
            nc.vector.memset(inv_din[:], 1.0 / DIN)
            z_ps = ppool.tile([1, CS], f32, tag="z")
            nc.tensor.matmul(z_ps[:], inv_din[:], en_sum[:])
            invz = misc.tile([1, CS], f32, tag="invz")
            nc.vector.reciprocal(invz[:], z_ps[:])
            # Broadcast 1/Z to all 32 batch partitions with a rank-1 matmul.
            ones_b = misc.tile([1, B], f32, tag="ones_b")
            nc.vector.memset(ones_b[:], 1.0)
            bc_ps = ppool.tile([B, CS], f32, tag="bc")
            nc.tensor.matmul(bc_ps[:], ones_b[:], invz[:])
            bc_sb = misc.tile([B, CS], f32, tag="bc_sb")
            nc.scalar.copy(bc_sb[:], bc_ps[:])

            # s = s_unnorm / Z ; squash: out = sqrt(ss)/(1+ss) * s
            sn = misc.tile([B, FW], f32, tag="sn")
            nc.vector.tensor_mul(
                sn[:].rearrange("p (c j) -> p c j", c=CS, j=DOUT),
                s_ps[:].rearrange("p (c j) -> p c j", c=CS, j=DOUT),
                bc_sb[:].unsqueeze(2).broadcast_to([B, CS, DOUT]),
            )
            sq = misc.tile([B, FW], f32, tag="sq")
            nc.vector.tensor_mul(sq[:], sn[:], sn[:])
            ss = misc.tile([B, CS], f32, tag="ss")
            nc.vector.reduce_sum(
                out=ss[:],
                in_=sq[:].rearrange("p (c j) -> p c j", c=CS, j=DOUT),
                axis=AxX,
            )
            eps_t = misc.tile([B, 1], f32, tag="eps")
            nc.vector.memset(eps_t[:], EPS)
            sqrt_ss = misc.tile([B, CS], f32, tag="sqrt_ss")
            nc.scalar.activation(sqrt_ss[:], ss[:], Sqrt, bias=eps_t[:])
            den = misc.tile([B, CS], f32, tag="den")
            nc.vector.tensor_scalar_add(den[:], ss[:], 1.0 + EPS)
            rden = misc.tile([B, CS], f32, tag="rden")
            nc.vector.reciprocal(rden[:], den[:])
            scl = misc.tile([B, CS], f32, tag="scl")
            nc.vector.tensor_mul(scl[:], sqrt_ss[:], rden[:])
            o_t = misc.tile([B, FW], f32, tag="o")
            nc.vector.tensor_mul(
                o_t[:].rearrange("p (c j) -> p c j", c=CS, j=DOUT),
                sn[:].rearrange("p (c j) -> p c j", c=CS, j=DOUT),
                scl[:].unsqueeze(2).broadcast_to([B, CS, DOUT]),
            )
            nc.sync.dma_start(out=out[:], in_=o_t[:])

    nc.compile()
    return nc


def _prep(x, W, R):
    """Layout-only host prep: shard + transpose into DMA-friendly tiles.

    Row index everywhere: p = n_in_blk * DIN + k.
    """
    x = np.ascontiguousarray(x, dtype=np.float32)
    W = np.ascontiguousarray(W, dtype=np.float32)
    R = np.ascontiguousarray(R, dtype=np.float32)

    # x_prep[p, blk*B + b] = x[b, n(blk, p), k(p)]   (shared by all cores)
    x_prep = np.ascontiguousarray(
        x.reshape(B, NBLK, NPB, DIN).transpose(2, 3, 1, 0).reshape(P, NBLK * B)
    )

    w_maps, r_maps = [], []
    for i in range(NCORES):
        cs = slice(i * CS, (i + 1) * CS)
        # w_prep[p, blk*FW + c*DOUT + j] = W[0, n(blk,p), c, j, k(p)]
        Wc = W[0][:, cs]  # [N, CS, DOUT, DIN]
        w_maps.append(
            np.ascontiguousarray(
                Wc.reshape(NBLK, NPB, CS, DOUT, DIN)
                .transpose(1, 4, 0, 2, 3)
                .reshape(P, NBLK * FW)
            )
        )
        # r_rep[p, blk*CS + c] = R[0, n(blk,p), c]   (replicated over k)
        Rc = R[0][:, cs].reshape(NBLK, NPB, CS).transpose(1, 0, 2)
        r_maps.append(
            np.ascontiguousarray(
                np.broadcast_to(Rc[:, None], (NPB, DIN, NBLK, CS)).reshape(
                    P, NBLK * CS
                )
            )
        )
    return x_prep, w_maps, r_maps


def kernel(**inputs):
    global _compiled, LAST_EXEC_TIME_NS
    x, W, R = inputs["x"], inputs["W"], inputs["R"]
    if _compiled is None:
        _compiled = _build()
    nc = _compiled

    x_prep, w_maps, r_maps = _prep(np.asarray(x), np.asarray(W), np.asarray(R))
    in_maps = [
        {"w_prep": w_maps[i], "x_prep": x_prep, "r_rep": r_maps[i]}
        for i in range(NCORES)
    ]

    from concourse.bass_utils import run_bass_kernel_spmd

    trace = bool(os.environ.get("BASS_KERNEL_TRACE"))
    res = run_bass_kernel_spmd(nc, in_maps, list(range(NCORES)), trace=trace)
    LAST_EXEC_TIME_NS = res.exec_time_ns

    outs = [res.results[i]["out"].reshape(B, CS, DOUT) for i in range(NCORES)]
    return np.ascontiguousarray(np.concatenate(outs, axis=1))
